# revision 1
# baseline (speedup 1.0000x reference)
"""Ragged-batch dual single-head attention (AttentionLayer) for Trainium2, 8 NeuronCores.

Data-parallel over graphs: 16 graphs per core, contiguous node segments
(batch_ids sorted). DMA-lean restructure (the original formulation was
DMA-bound at ~33MB/core).

Algebra (single head, one query per graph g, layer a in {0,1}):
  energy[n,(g,a)] = x[n] . qt_a[g] / sqrt(768),  qt_a = (Q_a @ kw_a) host-folded
      (Q_a = relu(gene/bionic @ fc_a^T + b) @ qw_a^T + qb_a; the Q.kb const
       cancels in softmax)
  pt = exp(energy) * mask               (node-major [128n, 4j, 32ga])
  ctx[(g,a), :] = sum_n pt[n,(g,a)] [x[n,:], 1]   (unnormalized; the appended
      ones column makes ctx[:, 768] the softmax denominator l)
  outU_a = ctx_a @ (ow_a @ vw_a)^T      (Wc_a host-folded)
  host: out = outU_0/l_0 + outU_1/l_1 + (vb@ow^T + ob folded bias)

Per-core HBM traffic ~12MB: X is loaded twice in fp8-e3m4 (hid-major for the
energy stationary, node-major for ctx — on-chip transposition is slower than
the extra DMA), all per-graph/per-layer weight matmuls are host-folded into
qt/Wc. The ctx accumulation lags the energy pass by two supertiles so the PE
instruction stream never waits head-of-line on the exp/mask chain; scratch
matmuls warm the PE HAM clock gate at the start and across the tail
transition; the 2.3MB Wc load is drip-fed in 196KB slices from supertile 3 on
so it never starves the X stream.
"""

import os
from contextlib import ExitStack

import numpy as np
import ml_dtypes

import concourse.bass as bass
import concourse.tile as tile
from concourse import bacc, mybir
from concourse.bass_utils import run_bass_kernel_spmd
from concourse.masks import make_identity

BF16 = ml_dtypes.bfloat16
F8E3 = ml_dtypes.float8_e3m4
F8E4 = ml_dtypes.float8_e4m3
HID = 768
HIDX = 776        # 768 x-features + ones column + 7 pad
GENE = 512
B = 128
NCORES = 8
G = B // NCORES   # graphs per core
GA = 2 * G        # query columns per core (2 layers x 16 graphs)
T = 512           # nodes per supertile
SCALE = 1.0 / float(np.sqrt(HID))

_BUILD_CACHE = {}


def _build(NJ, num_devices=NCORES):
    """NJ = number of valid 128-node chunks (global max, rounded up)."""
    ns = (NJ + 3) // 4
    jns = [min(4, NJ - 4 * t) for t in range(ns)]  # valid j-chunks per supertile
    dt = mybir.dt
    BF = dt.bfloat16
    F32 = dt.float32
    E3 = dt.float8e3
    E4 = dt.float8e4

    nc = bacc.Bacc("TRN2", target_bir_lowering=False, debug=False, num_devices=num_devices)

    xt_e = nc.declare_dram_parameter("xt4", [ns, 128, 4, 6, 128], E3, isOutput=False)
    xn_e = nc.declare_dram_parameter("xn4", [ns, 128, 4, HIDX], E3, isOutput=False)
    bid_e = nc.declare_dram_parameter("bid4", [128, ns, 4], F32, isOutput=False)
    io_e = nc.declare_dram_parameter("iota", [128, GA], BF, isOutput=False)
    qt_e = nc.declare_dram_parameter("qt", [128, 6, GA], BF, isOutput=False)
    wc_e = nc.declare_dram_parameter("wcT", [128, 12, HID], BF, isOutput=False)
    out_e = nc.declare_dram_parameter("out", [G, 2, HID], F32, isOutput=True)
    l_e = nc.declare_dram_parameter("lden", [GA, 1], F32, isOutput=True)

    with tile.TileContext(nc) as tc, ExitStack() as ctx:
        wpool = ctx.enter_context(tc.tile_pool(name="weights", bufs=1))
        apool = ctx.enter_context(tc.tile_pool(name="tail", bufs=1))
        xtp = ctx.enter_context(tc.tile_pool(name="xt", bufs=6))
        xnp = ctx.enter_context(tc.tile_pool(name="xn", bufs=6))
        ptp = ctx.enter_context(tc.tile_pool(name="pt", bufs=3))
        ps_e = ctx.enter_context(tc.tile_pool(name="ps_e", bufs=2, space="PSUM"))
        ps_acc = ctx.enter_context(tc.tile_pool(name="ps_acc", bufs=1, space="PSUM"))
        ps_t = ctx.enter_context(tc.tile_pool(name="ps_t", bufs=1, space="PSUM"))

        AFT = mybir.ActivationFunctionType

        # --- small preloads (qt gates the pipeline: issue first) ---
        qt_sb = wpool.tile([128, 6, GA], BF)
        nc.sync.dma_start(qt_sb[:], qt_e.ap())
        bid_sb = wpool.tile([128, ns, 4], F32)
        nc.sync.dma_start(bid_sb[:], bid_e.ap())
        io_sb = wpool.tile([128, GA], BF)
        nc.sync.dma_start(io_sb[:], io_e.ap())

        zbias = wpool.tile([128, 1], F32)
        nc.vector.memset(zbias[:], 0.0)

        # PE warm-up: the HAM clock gate holds the PE at 1.2 GHz until it has
        # been busy ~3.4us. The first X tile lands ~9-10us in, so burn scratch
        # matmuls now and run the real pipeline at 2.4 GHz from the start.
        scr = wpool.tile([128, 256], BF)
        nc.vector.memset(scr[:], 0.0)
        ident = wpool.tile([32, 32], BF)
        make_identity(nc, ident)
        warm_ps = ps_t.tile([32, 256], F32, tag="warm")
        for _ in range(18):
            nc.tensor.matmul(warm_ps[:], scr[:, 0:32], scr[:], start=True, stop=True)

        # Wc^T drip-fed one 196KB slice per supertile (needed only at the tail;
        # a single burst would starve the X stream mid-loop)
        wc_sb = wpool.tile([128, 12, HID], BF)

        # --- main loop: energy(t) | ctx(t-2) software pipeline ---
        ctxA = ps_acc.tile([GA, 384], F32)   # ctx cols 0:384 (hid)
        ctxB = ps_acc.tile([GA, 392], F32)   # ctx cols 384:768 + l + pad

        pend = []  # (pt, xn_t, t) awaiting ctx accumulation
        wc_fed = 0

        def ctx_step(pt, xn_t, t):
            jn = jns[t]
            for j in range(jn):
                nc.tensor.matmul(
                    ctxA[:], pt[:, j, :], xn_t[:, j, 0:384],
                    start=(t == 0 and j == 0), stop=(t == ns - 1 and j == jn - 1),
                )
                nc.tensor.matmul(
                    ctxB[:], pt[:, j, :], xn_t[:, j, 384:HIDX],
                    start=(t == 0 and j == 0), stop=(t == ns - 1 and j == jn - 1),
                )

        xt_tiles = {}

        def fetch_xt(t):
            jn = jns[t]
            xt_t = xtp.tile([128, 4, 6, 128], E3)
            if t == 0 and jn > 1:
                nc.sync.dma_start(xt_t[:, 0:1, :, :], xt_e.ap()[t][:, 0:1, :, :])
                nc.sync.dma_start(xt_t[:, 1:jn, :, :], xt_e.ap()[t][:, 1:jn, :, :])
            else:
                nc.sync.dma_start(xt_t[:, 0:jn, :, :], xt_e.ap()[t][:, 0:jn, :, :])
            xt_tiles[t] = xt_t

        fetch_xt(0)
        if ns > 1:
            fetch_xt(1)
        for t in range(2, min(4, ns)):
            fetch_xt(t)
        for t in range(ns):
            jn = jns[t]
            xt_t = xt_tiles.pop(t)
            xn_t = xnp.tile([128, 4, HIDX], E3)
            nc.sync.dma_start(xn_t[:, 0:jn, :], xn_e.ap()[t][:, 0:jn, :])
            if t + 4 < ns:
                fetch_xt(t + 4)
            if t >= 3:
                for _ in range(2):
                    if wc_fed < 12:
                        nc.sync.dma_start(wc_sb[:, wc_fed:wc_fed + 1, :], wc_e.ap()[:, wc_fed:wc_fed + 1, :])
                        wc_fed += 1

            et = ps_e.tile([128, 4, GA], F32)
            for j in range(jn):
                for c in range(6):
                    nc.tensor.matmul(
                        et[:, j, :],
                        xt_t[:, j, c, :],
                        qt_sb[:, c, :],
                        start=(j == 0 and c == 0), stop=(j == jn - 1 and c == 5),
                    )
            if t == 2 or t == ns - 1:
                # keep-warm burst right after energy(t): covers the two PE
                # stalls (pipeline fill, drain handoff) that otherwise trip
                # the HAM re-throttle; pinned on xt_t, off the exp/mask chain
                for _ in range(10):
                    nc.tensor.matmul(warm_ps[:], xt_t[:, 0, 0, 0:32], scr[:], start=True, stop=True)
            if len(pend) == 2:
                ctx_step(*pend.pop(0))
            pexp = ptp.tile([128, 4, GA], BF, tag="pexp")
            nc.scalar.activation(pexp[:, 0:jn, :], et[:, 0:jn, :], AFT.Exp, bias=zbias[:], scale=SCALE)
            msk = ptp.tile([128, 4, GA], BF, tag="msk")
            for j in range(jn):
                nc.vector.tensor_scalar(
                    msk[:, j, :], io_sb[:], bid_sb[:, t, j:j + 1], None,
                    op0=mybir.AluOpType.is_equal,
                )
            pt = ptp.tile([128, 4, GA], BF, tag="pt")
            nc.vector.tensor_mul(pt[:, 0:jn, :], pexp[:, 0:jn, :], msk[:, 0:jn, :])
            pend.append((pt, xn_t, t))

        while wc_fed < 12:
            nc.sync.dma_start(wc_sb[:, wc_fed:wc_fed + 1, :], wc_e.ap()[:, wc_fed:wc_fed + 1, :])
            wc_fed += 1
        pt_last = pend[-1][0]
        for args in pend:
            ctx_step(*args)

        # keep the PE busy (and the HAM gate warm) while the tail copies and
        # transposes drain; these depend on the last pt so they schedule here
        for _ in range(14):
            nc.tensor.matmul(warm_ps[:], pt_last[:, 0, :], scr[:], start=True, stop=True)

        # --- tail: ctx -> (DVE quadrant transpose) -> Wc projection ---
        l_sb = apool.tile([GA, 1], F32)
        nc.vector.tensor_copy(l_sb[:], ctxB[:, 384:385])
        nc.sync.dma_start(l_e.ap(), l_sb[:])

        # ctxT[p, k, ga] = ctx[ga, k*128 + p] via PE transpose-mode (the DVE
        # StreamTranspose path measured ~4us on the strided views; PE does it
        # in ~1.7us while it would otherwise idle).
        ctx_sb = apool.tile([GA, HID], BF)
        nc.scalar.activation(ctx_sb[:, 0:384], ctxA[:], AFT.Identity, bias=zbias[0:GA], scale=1.0)
        nc.vector.tensor_copy(ctx_sb[:, 384:768], ctxB[:, 0:384])
        tp = ps_t.tile([128, 6, GA], BF, tag="tp")
        for k in range(6):
            nc.tensor.transpose(tp[:, k, :], ctx_sb[:, k * 128:(k + 1) * 128], ident[:])
        ctxT = apool.tile([128, 6, GA], BF)
        nc.vector.tensor_copy(ctxT[:], tp[:])

        out_sb = apool.tile([G, 2, HID], F32)
        for a in range(2):
            oA = ps_t.tile([G, 384], F32, tag="oA")
            oB = ps_t.tile([G, 384], F32, tag="oB")
            for k in range(6):
                nc.tensor.matmul(
                    oA[:], ctxT[:, k, a * G:(a + 1) * G], wc_sb[:, a * 6 + k, 0:384],
                    start=(k == 0), stop=(k == 5),
                )
                nc.tensor.matmul(
                    oB[:], ctxT[:, k, a * G:(a + 1) * G], wc_sb[:, a * 6 + k, 384:768],
                    start=(k == 0), stop=(k == 5),
                )
            nc.scalar.activation(out_sb[:, a, 0:384], oA[:], AFT.Identity, bias=zbias[0:G], scale=1.0)
            nc.vector.tensor_copy(out_sb[:, a, 384:768], oB[:])
            nc.sync.dma_start(out_e.ap()[:, a, :], out_sb[:, a, :])

    nc.compile()
    return nc


def _host_qt(g_in, fcw, fcb, qw, qb, kw):
    g = np.maximum(g_in.astype(np.float32) @ fcw.T + fcb, 0.0)
    Q = g @ qw.T + qb
    return Q @ kw  # [B, HID]; energy = qt . x (Q.kb const cancels in softmax)


def _prep_inputs(x, batch_ids, gene, bionic, p):
    bids = np.asarray(batch_ids).astype(np.int64)
    x = np.asarray(x, dtype=np.float32)

    bounds = np.searchsorted(bids, np.arange(0, B + 1, G))
    counts = np.diff(bounds)
    NJ = max((int(counts.max()) + 127) // 128, 1)
    ns = (NJ + 3) // 4
    C = ns * T  # tile-padded capacity (zero-filled beyond NJ*128)

    f32 = np.float32
    qts = [
        _host_qt(np.asarray(gene, f32), np.asarray(p["fc0_w"], f32), np.asarray(p["fc0_b"], f32),
                 np.asarray(p["a0_qw"], f32), np.asarray(p["a0_qb"], f32), np.asarray(p["a0_kw"], f32)),
        _host_qt(np.asarray(bionic, f32), np.asarray(p["fc1_w"], f32), np.asarray(p["fc1_b"], f32),
                 np.asarray(p["a1_qw"], f32), np.asarray(p["a1_qb"], f32), np.asarray(p["a1_kw"], f32)),
    ]
    wcT_parts = []
    for a in range(2):
        wc = np.asarray(p[f"a{a}_ow"], f32) @ np.asarray(p[f"a{a}_vw"], f32)  # [768o, 768h]
        wcT_parts.append(wc.T.reshape(6, 128, HID).transpose(1, 0, 2))        # [128, 6, 768]
    wcT = np.ascontiguousarray(np.concatenate(wcT_parts, axis=1)).astype(BF16)  # [128, 12, 768]

    out_bias = (
        np.asarray(p["a0_vb"], f32) @ np.asarray(p["a0_ow"], f32).T + np.asarray(p["a0_ob"], f32)
        + np.asarray(p["a1_vb"], f32) @ np.asarray(p["a1_ow"], f32).T + np.asarray(p["a1_ob"], f32)
    )

    iota_pb = np.ascontiguousarray(
        np.broadcast_to(np.tile(np.arange(G, dtype=np.float32), 2), (128, GA))
    ).astype(BF16)                          # [128, GA]: col ga -> ga % 16
    in_maps = []
    for c in range(NCORES):
        s, e = int(bounds[c]), int(bounds[c + 1])
        cnt = e - s
        xs = np.zeros((C, HID), f32)
        xs[:cnt] = x[s:e]
        xt4 = np.ascontiguousarray(
            xs.T.reshape(6, 128, ns, 4, 128).transpose(2, 1, 3, 0, 4)
        ).astype(F8E3)                      # [ns, 128, 4(j), 6(c), 128]
        xsx = np.zeros((C, HIDX), f32)
        xsx[:, :HID] = xs
        xsx[:, HID] = 1.0                   # ones column -> softmax denominator
        xn4 = np.ascontiguousarray(
            xsx.reshape(ns, 4, 128, HIDX).transpose(0, 2, 1, 3)
        ).astype(F8E3)                      # [ns, 128, 4(j), 776]

        lab = np.full((C,), 255.0, np.float32)
        lab[:cnt] = (bids[s:e] - c * G).astype(np.float32)
        bid4 = np.ascontiguousarray(
            lab.reshape(ns, 4, 128).transpose(2, 0, 1)
        ).astype(np.float32)                # [128, ns, 4(j)]

        qcat = np.concatenate([qts[0][c * G:(c + 1) * G].T, qts[1][c * G:(c + 1) * G].T], axis=1)  # [768, 32]
        qt_pb = np.ascontiguousarray(qcat.reshape(6, 128, GA).transpose(1, 0, 2)).astype(BF16)

        in_maps.append({
            "xt4": xt4,
            "xn4": xn4,
            "bid4": bid4,
            "iota": iota_pb,
            "qt": qt_pb,
            "wcT": wcT,
        })
    return in_maps, NJ, out_bias


def kernel(**inputs):
    x = inputs["x"]
    batch_ids = inputs["batch_ids"]
    gene = inputs["gene"]
    bionic = inputs["bionic"]
    in_maps, NJ, out_bias = _prep_inputs(x, batch_ids, gene, bionic, inputs)

    if NJ not in _BUILD_CACHE:
        _BUILD_CACHE[NJ] = _build(NJ)
    nc = _BUILD_CACHE[NJ]

    prof_dir = os.environ.get("BASSK_PROFILE_DIR")
    if prof_dir:
        from trn_agent_boot.trn_boot import _ntff_profile_via_ctypes
        hook = _ntff_profile_via_ctypes("/opt/axon/libaxon_pjrt.so")
        os.makedirs(prof_dir, exist_ok=True)
        with hook(prof_dir, [0]):
            res = run_bass_kernel_spmd(nc, in_maps, core_ids=list(range(NCORES)))
        kernel.last_nc = nc
    else:
        res = run_bass_kernel_spmd(nc, in_maps, core_ids=list(range(NCORES)))

    out = np.empty((B, HID), np.float32)
    for c in range(NCORES):
        ou = np.asarray(res.results[c]["out"], np.float32)        # [G, 2, 768]
        l32 = np.asarray(res.results[c]["lden"], np.float32)[:, 0]  # [32]
        l0 = l32[:G, None]
        l1 = l32[G:, None]
        acc = np.zeros((G, HID), np.float32)
        np.divide(ou[:, 0, :], l0, out=acc, where=l0 > 0)
        tmp = np.zeros((G, HID), np.float32)
        np.divide(ou[:, 1, :], l1, out=tmp, where=l1 > 0)
        out[c * G:(c + 1) * G] = acc + tmp + out_bias
    return out



# revision 3
# speedup vs baseline: 1.1778x; 1.1778x over previous
"""Ragged-batch dual single-head attention (AttentionLayer) for Trainium2, 8 NeuronCores.

Data-parallel over graphs: 16 graphs per core, contiguous node segments
(batch_ids sorted).

Algebra (single head, one query per graph g, layer a in {0,1}):
  energy[n,(g,a)] = x[n] . qt_a[g] / sqrt(768),  qt_a = (Q_a @ kw_a) host-folded
      (Q_a = relu(gene/bionic @ fc_a^T + b) @ qw_a^T + qb_a; the Q.kb const
       cancels in softmax)
  pt = exp(energy) * mask               (node-major [128n, 4j, 32ga])
  ctx[(g,a), :] = sum_n pt[n,(g,a)] [x[n,:], 1]   (unnormalized; the appended
      ones column makes ctx[:, 768] the softmax denominator l)
  host: out_a = (ctx_a[:, :768] / ctx_a[:, 768]) @ (ow_a @ vw_a)^T, plus the
      folded bias vb@ow^T + ob.  The [256,768]x[768,768] tail projection runs
      on host (like the softmax division), so the device only streams X.

Per-core HBM traffic ~9.6MB: X is loaded twice in fp8-e3m4 (hid-major for the
energy stationary, node-major for ctx — on-chip transposition costs a third
PE pass over X, slower than the extra DMA). DMA triggers cost ~610ns each on
the issuing engine queue, so the two X streams are issued from different
queues (xt on sync, xn on gpsimd) and the small preloads from scalar/vector;
a short scratch-matmul burst warms the PE HAM clock gate before the first X
tile lands.
"""

import os
from contextlib import ExitStack

import numpy as np
import ml_dtypes

import concourse.bass as bass
import concourse.tile as tile
from concourse import bacc, mybir
from concourse.bass_utils import run_bass_kernel_spmd

BF16 = ml_dtypes.bfloat16
F8E3 = ml_dtypes.float8_e3m4
HID = 768
HIDX = 776        # 768 x-features + ones column + 7 pad
GENE = 512
B = 128
NCORES = 8
G = B // NCORES   # graphs per core
GA = 2 * G        # query columns per core (2 layers x 16 graphs)
T = 512           # nodes per supertile
SCALE = 1.0 / float(np.sqrt(HID))

_BUILD_CACHE = {}


def _build(NJ, num_devices=NCORES):
    """NJ = number of valid 128-node chunks (global max, rounded up)."""
    ns = (NJ + 3) // 4
    jns = [min(4, NJ - 4 * t) for t in range(ns)]  # valid j-chunks per supertile
    dt = mybir.dt
    BF = dt.bfloat16
    F32 = dt.float32
    E3 = dt.float8e3

    nc = bacc.Bacc("TRN2", target_bir_lowering=False, debug=False, num_devices=num_devices)

    xt_e = nc.declare_dram_parameter("xt4", [ns, 128, 4, 6, 128], E3, isOutput=False)
    xn_e = nc.declare_dram_parameter("xn4", [ns, 128, 4, HIDX], E3, isOutput=False)
    bid_e = nc.declare_dram_parameter("bid4", [128, ns, 4], F32, isOutput=False)
    io_e = nc.declare_dram_parameter("iota", [128, GA], BF, isOutput=False)
    qt_e = nc.declare_dram_parameter("qt", [128, 6, GA], BF, isOutput=False)
    ctx_e = nc.declare_dram_parameter("ctx", [GA, HIDX], F32, isOutput=True)

    with tile.TileContext(nc) as tc, ExitStack() as ctx:
        wpool = ctx.enter_context(tc.tile_pool(name="weights", bufs=1))
        xtp = ctx.enter_context(tc.tile_pool(name="xt", bufs=6))
        xnp = ctx.enter_context(tc.tile_pool(name="xn", bufs=6))
        ptp = ctx.enter_context(tc.tile_pool(name="pt", bufs=3))
        ps_e = ctx.enter_context(tc.tile_pool(name="ps_e", bufs=2, space="PSUM"))
        ps_acc = ctx.enter_context(tc.tile_pool(name="ps_acc", bufs=1, space="PSUM"))
        ps_w = ctx.enter_context(tc.tile_pool(name="ps_w", bufs=1, space="PSUM"))

        AFT = mybir.ActivationFunctionType

        xt_tiles = {}

        def fetch_xt(t):
            jn = jns[t]
            xt_t = xtp.tile([128, 4, 6, 128], E3)
            if t == 0 and jn > 1:
                # split so the first 128-node chunk lands (and energy starts) sooner
                nc.sync.dma_start(xt_t[:, 0:1, :, :], xt_e.ap()[t][:, 0:1, :, :])
                nc.sync.dma_start(xt_t[:, 1:jn, :, :], xt_e.ap()[t][:, 1:jn, :, :])
            else:
                nc.sync.dma_start(xt_t[:, 0:jn, :, :], xt_e.ap()[t][:, 0:jn, :, :])
            xt_tiles[t] = xt_t

        xn_tiles = {}

        def fetch_xn(t):
            jn = jns[t]
            xn_t = xnp.tile([128, 4, HIDX], E3)
            nc.gpsimd.dma_start(xn_t[:, 0:jn, :], xn_e.ap()[t][:, 0:jn, :])
            xn_tiles[t] = xn_t

        # X triggers first — they gate the whole pipeline; the small preloads
        # go out concurrently on the scalar/vector queues.
        fetch_xt(0)
        fetch_xn(0)
        qt_sb = wpool.tile([128, 6, GA], BF)
        nc.scalar.dma_start(qt_sb[:], qt_e.ap())
        scr = wpool.tile([128, 256], BF)
        nc.vector.memset(scr[:], 0.0)
        bid_sb = wpool.tile([128, ns, 4], F32)
        nc.scalar.dma_start(bid_sb[:], bid_e.ap())
        io_sb = wpool.tile([128, GA], BF)
        nc.scalar.dma_start(io_sb[:], io_e.ap())

        # PE warm-up: the HAM clock gate holds the PE at 1.2 GHz until it has
        # been busy a while; burn scratch matmuls so the real pipeline starts
        # closer to 2.4 GHz.
        warm_ps = ps_w.tile([32, 256], F32, tag="warm")
        for _ in range(10):
            nc.tensor.matmul(warm_ps[:], scr[:, 0:32], scr[:], start=True, stop=True)

        for t in range(1, min(5, ns)):
            fetch_xt(t)
        for t in range(1, min(3, ns)):
            fetch_xn(t)

        # --- main loop: energy(t) | ctx(t-2) software pipeline ---
        ctxA = ps_acc.tile([GA, 384], F32)   # ctx cols 0:384 (hid)
        ctxB = ps_acc.tile([GA, 392], F32)   # ctx cols 384:768 + l + pad

        pend = []  # (pt, xn_t, t) awaiting ctx accumulation

        def ctx_step(pt, xn_t, t):
            jn = jns[t]
            for j in range(jn):
                nc.tensor.matmul(
                    ctxA[:], pt[:, j, :], xn_t[:, j, 0:384],
                    start=(t == 0 and j == 0), stop=(t == ns - 1 and j == jn - 1),
                )
                nc.tensor.matmul(
                    ctxB[:], pt[:, j, :], xn_t[:, j, 384:HIDX],
                    start=(t == 0 and j == 0), stop=(t == ns - 1 and j == jn - 1),
                )

        for t in range(ns):
            jn = jns[t]
            xt_t = xt_tiles.pop(t)
            if t + 5 < ns:
                fetch_xt(t + 5)
            if t + 3 < ns:
                fetch_xn(t + 3)

            et = ps_e.tile([128, 4, GA], F32)
            for j in range(jn):
                for c in range(6):
                    nc.tensor.matmul(
                        et[:, j, :],
                        xt_t[:, j, c, :],
                        qt_sb[:, c, :],
                        start=(j == 0 and c == 0), stop=(j == jn - 1 and c == 5),
                    )
            if len(pend) == 2:
                ctx_step(*pend.pop(0))
            pexp = ptp.tile([128, 4, GA], BF, tag="pexp")
            nc.scalar.activation(pexp[:, 0:jn, :], et[:, 0:jn, :], AFT.Exp, bias=0.0, scale=SCALE)
            msk = ptp.tile([128, 4, GA], BF, tag="msk")
            for j in range(jn):
                nc.vector.tensor_scalar(
                    msk[:, j, :], io_sb[:], bid_sb[:, t, j:j + 1], None,
                    op0=mybir.AluOpType.is_equal,
                )
            pt = ptp.tile([128, 4, GA], BF, tag="pt")
            nc.vector.tensor_mul(pt[:, 0:jn, :], pexp[:, 0:jn, :], msk[:, 0:jn, :])
            pend.append((pt, xn_tiles.pop(t), t))

        for args in pend:
            ctx_step(*args)

        # --- tail: ctx PSUM -> SBUF -> DRAM; projection happens on host ---
        ctx_sb = wpool.tile([GA, HIDX], F32)
        nc.scalar.activation(ctx_sb[:, 0:384], ctxA[:], AFT.Copy, bias=0.0, scale=1.0)
        nc.vector.tensor_copy(ctx_sb[:, 384:HIDX], ctxB[:])
        nc.scalar.dma_start(ctx_e.ap(), ctx_sb[:])

    nc.compile()
    return nc


def _host_qt(g_in, fcw, fcb, qw, qb, kw):
    g = np.maximum(g_in.astype(np.float32) @ fcw.T + fcb, 0.0)
    Q = g @ qw.T + qb
    return Q @ kw  # [B, HID]; energy = qt . x (Q.kb const cancels in softmax)


def _prep_inputs(x, batch_ids, gene, bionic, p):
    bids = np.asarray(batch_ids).astype(np.int64)
    x = np.asarray(x, dtype=np.float32)

    bounds = np.searchsorted(bids, np.arange(0, B + 1, G))
    counts = np.diff(bounds)
    NJ = max((int(counts.max()) + 127) // 128, 1)
    ns = (NJ + 3) // 4
    C = ns * T  # tile-padded capacity (zero-filled beyond NJ*128)

    f32 = np.float32
    qts = [
        _host_qt(np.asarray(gene, f32), np.asarray(p["fc0_w"], f32), np.asarray(p["fc0_b"], f32),
                 np.asarray(p["a0_qw"], f32), np.asarray(p["a0_qb"], f32), np.asarray(p["a0_kw"], f32)),
        _host_qt(np.asarray(bionic, f32), np.asarray(p["fc1_w"], f32), np.asarray(p["fc1_b"], f32),
                 np.asarray(p["a1_qw"], f32), np.asarray(p["a1_qb"], f32), np.asarray(p["a1_kw"], f32)),
    ]
    wcs = [
        np.asarray(p["a0_ow"], f32) @ np.asarray(p["a0_vw"], f32),  # [768o, 768h]
        np.asarray(p["a1_ow"], f32) @ np.asarray(p["a1_vw"], f32),
    ]

    out_bias = (
        np.asarray(p["a0_vb"], f32) @ np.asarray(p["a0_ow"], f32).T + np.asarray(p["a0_ob"], f32)
        + np.asarray(p["a1_vb"], f32) @ np.asarray(p["a1_ow"], f32).T + np.asarray(p["a1_ob"], f32)
    )

    iota_pb = np.ascontiguousarray(
        np.broadcast_to(np.tile(np.arange(G, dtype=np.float32), 2), (128, GA))
    ).astype(BF16)                          # [128, GA]: col ga -> ga % 16
    in_maps = []
    for c in range(NCORES):
        s, e = int(bounds[c]), int(bounds[c + 1])
        cnt = e - s
        xs = np.zeros((C, HID), f32)
        xs[:cnt] = x[s:e]
        xt4 = np.ascontiguousarray(
            xs.T.reshape(6, 128, ns, 4, 128).transpose(2, 1, 3, 0, 4)
        ).astype(F8E3)                      # [ns, 128, 4(j), 6(c), 128]
        xsx = np.zeros((C, HIDX), f32)
        xsx[:, :HID] = xs
        xsx[:, HID] = 1.0                   # ones column -> softmax denominator
        xn4 = np.ascontiguousarray(
            xsx.reshape(ns, 4, 128, HIDX).transpose(0, 2, 1, 3)
        ).astype(F8E3)                      # [ns, 128, 4(j), 776]

        lab = np.full((C,), 255.0, np.float32)
        lab[:cnt] = (bids[s:e] - c * G).astype(np.float32)
        bid4 = np.ascontiguousarray(
            lab.reshape(ns, 4, 128).transpose(2, 0, 1)
        ).astype(np.float32)                # [128, ns, 4(j)]

        qcat = np.concatenate([qts[0][c * G:(c + 1) * G].T, qts[1][c * G:(c + 1) * G].T], axis=1)  # [768, 32]
        qt_pb = np.ascontiguousarray(qcat.reshape(6, 128, GA).transpose(1, 0, 2)).astype(BF16)

        in_maps.append({
            "xt4": xt4,
            "xn4": xn4,
            "bid4": bid4,
            "iota": iota_pb,
            "qt": qt_pb,
        })
    return in_maps, NJ, out_bias, wcs


def kernel(**inputs):
    x = inputs["x"]
    batch_ids = inputs["batch_ids"]
    gene = inputs["gene"]
    bionic = inputs["bionic"]
    in_maps, NJ, out_bias, wcs = _prep_inputs(x, batch_ids, gene, bionic, inputs)

    if NJ not in _BUILD_CACHE:
        _BUILD_CACHE[NJ] = _build(NJ)
    nc = _BUILD_CACHE[NJ]

    prof_dir = os.environ.get("BASSK_PROFILE_DIR")
    if prof_dir:
        from trn_agent_boot.trn_boot import _ntff_profile_via_ctypes
        hook = _ntff_profile_via_ctypes("/opt/axon/libaxon_pjrt.so")
        os.makedirs(prof_dir, exist_ok=True)
        with hook(prof_dir, [0]):
            res = run_bass_kernel_spmd(nc, in_maps, core_ids=list(range(NCORES)))
        kernel.last_nc = nc
    else:
        res = run_bass_kernel_spmd(nc, in_maps, core_ids=list(range(NCORES)))

    out = np.empty((B, HID), np.float32)
    for c in range(NCORES):
        cx = np.asarray(res.results[c]["ctx"], np.float32)   # [32, 776]
        hid = cx[:, :HID]
        l = cx[:, HID]
        l0 = l[:G, None]
        l1 = l[G:, None]
        a0 = np.zeros((G, HID), np.float32)
        np.divide(hid[:G], l0, out=a0, where=l0 > 0)
        a1 = np.zeros((G, HID), np.float32)
        np.divide(hid[G:], l1, out=a1, where=l1 > 0)
        out[c * G:(c + 1) * G] = a0 @ wcs[0].T + a1 @ wcs[1].T + out_bias
    return out
